# revision 8
# baseline (speedup 1.0000x reference)
"""Trainium2 Bass kernel for nn_ModeONet: complex mode superposition.

Sharding: data-parallel over B — each of the 8 NeuronCores processes one
batch sample (its own mode_shapes/mode_responses/freqs slice).

Per-core math (b fixed; S = mode_shapes[b] as (64, 9216) with rows
[s_r(32p); s_i(32p)], T = mode_responses[b] transposed to (64, 128)):
  norm2[p]   = sum_{c,wh} S[c*32+p]^2
  v[k]       = 1/(sqrt(norm2)+1e-8) / 32            (1/P folded in)
  lhsT_r     = [ t_r * v ; -t_i * v ]   (64,128)
  lhsT_i     = [ t_i * v ;  t_r * v ]
  r = lhsT_r.T @ S, i = lhsT_i.T @ S                (TensorE)
  mag2 = r^2 + i^2; ln = (Ln(mag2+1e-8)+bias_f)/4.2
  fnorm = sqrt(max(sum_wh mag2, f32eps)); field = (r, i) * (1/fnorm)
  aux_b = mean_p ln(sqrt(sum_{f,c} T^2) + 1)        (host averages over b)

Matmuls run in bf16 with a hi/lo split for ~fp32 accuracy, 2 matmuls per
product: S is host-split into S_hi/S_lo stacked on partitions
(s2 = [S_hi; S_lo], (128, WH) bf16); lhsT is split on device.  Then
  MM1 (K=64):  a_hi.T @ S_hi
  MM2 (K=128): [a_lo; a_hi].T @ [S_hi; S_lo] = a_lo@S_hi + a_hi@S_lo
accumulated in PSUM (the dropped a_lo@S_lo term is ~2^-32).  The mode
norm is computed from s2 directly: sum(hi^2)+sum(lo^2) — the missing
2*sum(hi*lo) cross term is a random-rounding sum, relative size ~4e-6.

r and i for one column-pair live in ONE (128, 2048) PSUM tile so a single
ScalarE Square covers both, and its accum_out row-sum IS the fnorm
partial.  Pass 2 recomputes the matmuls (TensorE has slack) so the
mandatory PSUM->SBUF move doubles as the 1/fnorm scaling.  The bias
interpolation over the 250-entry table (1K elements) is host-computed.

The walrus in this environment caps sync waits at 1 per instruction,
while Tile emits as many as needed; _split_sync_waits hoists the excess
onto preceding same-engine NoOps (equivalent under per-engine program
order).
"""
import numpy as np
import ml_dtypes
import concourse.bass as bass
import concourse.bass_utils as _bu
import concourse.tile as tile
from concourse import mybir
from concourse.bass_utils import run_bass_kernel_spmd

def _dedup_ldweights(nc):
    """Drop InstLdweights that reload the stationary operand already resident
    in the PE array (identical tensor/offset/pattern as the previous LDW on
    the PE stream).  The pipeline's walrus runs with --enable-ldw-opt=false,
    so repeated weights would otherwise reload every matmul.  A dropped LDW
    that carries sync info is replaced by a NoOp (keeps waits/updates)."""
    n = 0
    for fn in nc.m.functions:
        for blk in fn.blocks:
            out = []
            last_sig = None
            for inst in blk.instructions:
                if getattr(inst, "engine", None) != mybir.EngineType.PE:
                    out.append(inst)
                    continue
                tname = type(inst).__name__
                if tname == "InstLdweights":
                    ap = inst.ins[0]
                    bap = getattr(ap, "bass_ap", None)
                    sig = (
                        (bap.tensor.name, bap.offset, str(bap.ap))
                        if bap is not None
                        else None
                    )
                    if sig is not None and sig == last_sig:
                        si = inst.sync_info
                        has_sync = si is not None and (si.on_wait or si.on_update)
                        if has_sync:
                            out.append(
                                mybir.InstNoOp(
                                    name=f"{inst.name}-ldwdedup",
                                    engine=inst.engine,
                                    ins=[],
                                    outs=[],
                                    bass_nofuse=True,
                                    sync_info=si,
                                )
                            )
                        n += 1
                        continue
                    last_sig = sig
                elif tname == "InstMatmult":
                    pass  # matmuls don't clobber the weight buffer
                else:
                    last_sig = None  # anything else on PE: be conservative
                out.append(inst)
            blk.instructions = out
    return n

B, F, P, W, H = 8, 128, 32, 96, 96
WH = W * H                     # 9216
K = 2 * P                      # 64 contraction rows (real; imag)
PAIR = 1024                    # column-pair width (r|i -> one 4-bank PSUM tile)
NPAIR = WH // PAIR             # 9
NORM_CHUNK = WH // 4           # 2304
AMPLITUDE_STD = 4.2
EPS = 1e-8
F32_EPS = float(np.finfo(np.float32).eps)
AF = mybir.ActivationFunctionType
ALU = mybir.AluOpType
DT = mybir.dt

# cst128 column layout
C_TT, C_TT2, C_PS4, C_SGN, C_W32 = 0, 128, 256, 384, 385


def _split_sync_waits(nc, inst_cap=1):
    """Hoist excess sync waits onto preceding same-engine NoOps."""
    for fn in nc.m.functions:
        for blk in fn.blocks:
            out = []
            for inst in blk.instructions:
                si = inst.sync_info
                waits = list(si.on_wait) if si is not None and si.on_wait else []
                if len(waits) > inst_cap and not isinstance(
                    inst, mybir.InstCall | mybir.InstUnconditionalBranch
                ):
                    keep = waits[-inst_cap:]
                    for ci, w in enumerate(waits[:-inst_cap]):
                        out.append(
                            mybir.InstNoOp(
                                name=f"{inst.name}-wsplit{ci}",
                                engine=inst.engine,
                                ins=[],
                                outs=[],
                                bass_nofuse=True,
                                sync_info=mybir.SyncInfo(on_wait=[w], on_update=[]),
                            )
                        )
                    inst.sync_info = mybir.SyncInfo(
                        on_wait=keep,
                        on_update=list(si.on_update) if si.on_update else [],
                    )
                out.append(inst)
            blk.instructions = out


def _emit_pair_matmuls(nc, ps, s16_t, l16_r, l16_i, j):
    """4 fp16 matmuls for column pair j -> one (128, 2048) PSUM tile [r | i].

    The two 512-column chunks of a pair run CONCURRENTLY as 2-way K=64
    row-tiles: chunk 0 on PE rows 0-63, chunk 1 on rows 64-127 (s16 and the
    coefficients are duplicated across partition halves)."""
    p_ri = ps.tile([F, 2 * PAIR], DT.float32, tag="mm")
    for base, l16 in ((0, l16_r), (PAIR, l16_i)):
        for h in range(2):
            cols = slice(j * PAIR + h * 512, j * PAIR + (h + 1) * 512)
            reg = slice(base + h * 512, base + (h + 1) * 512)
            rows = slice(h * K, (h + 1) * K)
            nc.tensor.matmul(
                p_ri[:, reg], l16[rows, :], s16_t[rows, cols],
                start=True, stop=True,
            )
    return p_ri


def build_nc():
    nc = bass.Bass("TRN2", target_bir_lowering=False, debug=False)
    s16_d = nc.dram_tensor("s16", (2 * K, WH), DT.float16, kind="ExternalInput").ap()
    cst_d = nc.dram_tensor("cst", (2 * K, 386), DT.float32, kind="ExternalInput").ap()
    biasf_d = nc.dram_tensor("biasf", (F, 2), DT.float32, kind="ExternalInput").ap()
    ln_d = nc.dram_tensor("ln", (F, WH), DT.float32, kind="ExternalOutput").ap()
    fld_d = nc.dram_tensor("fld", (F, 2, WH), DT.float32, kind="ExternalOutput").ap()
    aux_d = nc.dram_tensor("aux", (1, 1), DT.float32, kind="ExternalOutput").ap()

    with tile.TileContext(nc) as tc:
        with (
            tc.tile_pool(name="cp", bufs=1) as cp,
            tc.tile_pool(name="wp", bufs=3) as wp,
            tc.tile_pool(name="np2", bufs=2) as np2,
            tc.tile_pool(name="ps", bufs=2, space="PSUM") as ps,
        ):
            cst = cp.tile([2 * K, 386], DT.float32, tag="cst")
            nc.sync.dma_start(cst[:], cst_d)
            biasf = cp.tile([F, 2], DT.float32, tag="biasf")
            nc.sync.dma_start(biasf[:], biasf_d)
            s16_t = cp.tile([2 * K, WH], DT.float16, tag="s16")
            for q in range(4):
                cs = slice(q * NORM_CHUNK, (q + 1) * NORM_CHUNK)
                nc.sync.dma_start(s16_t[0:K, cs], s16_d[0:K, cs])
            nc.sync.dma_start(s16_t[K : 2 * K, :], s16_d[K : 2 * K, :])

            tt = cst[0:K, C_TT : C_TT + 128]
            tt2 = cst[0:K, C_TT2 : C_TT2 + 128]
            ps4_128 = cst[:, C_PS4 : C_PS4 + 128]
            ps4_64 = cst[0:K, C_PS4 : C_PS4 + 128]
            sgn = cst[:, C_SGN : C_SGN + 1]
            w32 = cst[0:K, C_W32 : C_W32 + 1]
            epsb = biasf[:, 1:2]

            # ---- mode norms: red[k] = sum_wh s2[k]^2 over all 128 rows ----
            red4 = []
            for q in range(4):
                cs = slice(q * NORM_CHUNK, (q + 1) * NORM_CHUNK)
                scr = np2.tile([K, NORM_CHUNK], DT.float32, tag="nscr")
                rq = cp.tile([K, 1], DT.float32, tag=f"red{q}")
                if q < 3:
                    nc.scalar.activation(
                        scr[:], s16_t[0:K, cs], AF.Square, accum_out=rq[:]
                    )
                else:
                    nc.vector.tensor_mul(scr[:], s16_t[0:K, cs], s16_t[0:K, cs])
                    nc.vector.tensor_reduce(
                        rq[:], scr[:], axis=mybir.AxisListType.X, op=ALU.add
                    )
                red4.append(rq)
            r01 = cp.tile([K, 1], DT.float32, tag="r01")
            nc.vector.tensor_add(r01[:], red4[0][:], red4[1][:])
            r23 = cp.tile([K, 1], DT.float32, tag="r23")
            nc.vector.tensor_add(r23[:], red4[2][:], red4[3][:])
            red_s = cp.tile([K, 1], DT.float32, tag="red_s")
            nc.vector.tensor_add(red_s[:], r01[:], r23[:])

            # pair-sum across (real, imag) rows k = p mod 32 via PE (K=64)
            n2_ps = ps.tile([F, 1], DT.float32, tag="mm")
            nc.tensor.matmul(n2_ps[:], ps4_64, red_s[:], start=True, stop=True)
            # v = n2^(-1/2) = Exp(-0.5*Ln(n2)); the reference's +1e-8 on the
            # norm is ~1e-10 relative here (norm ~ O(100)) and is dropped.
            lnn = cp.tile([F, 1], DT.float32, tag="lnn")
            nc.scalar.activation(lnn[:], n2_ps[:], AF.Ln)
            v = cp.tile([F, 1], DT.float32, tag="v")
            nc.scalar.activation(v[:], lnn[:], AF.Exp, scale=-0.5)
            vr = cp.tile([2 * K, 1], DT.float32, tag="vr")
            nc.vector.tensor_mul(vr[:], v[:], sgn)

            # lhsT in fp32 on all 128 partitions (rows duplicated), then split
            lr32 = cp.tile([2 * K, F], DT.float32, tag="lr32")
            nc.vector.tensor_scalar(
                out=lr32[:], in0=cst[:, C_TT : C_TT + 128], scalar1=vr[:],
                scalar2=None, op0=ALU.mult,
            )
            li32 = cp.tile([2 * K, F], DT.float32, tag="li32")
            nc.vector.tensor_scalar(
                out=li32[:], in0=cst[:, C_TT2 : C_TT2 + 128], scalar1=v[:],
                scalar2=None, op0=ALU.mult,
            )
            l16_r = cp.tile([2 * K, F], DT.float16, tag="l16r")
            nc.vector.tensor_copy(l16_r[:], lr32[:])
            l16_i = cp.tile([2 * K, F], DT.float16, tag="l16i")
            nc.vector.tensor_copy(l16_i[:], li32[:])

            # ---- aux sparsity scalar ----
            tsq = cp.tile([K, F], DT.float32, tag="tsq")
            redt = cp.tile([K, 1], DT.float32, tag="redt")
            nc.scalar.activation(tsq[:], tt, AF.Square, accum_out=redt[:])
            n2t_ps = ps.tile([F, 1], DT.float32, tag="mm")
            nc.tensor.matmul(n2t_ps[:], ps4_64, redt[:], start=True, stop=True)
            lnt2 = cp.tile([F, 1], DT.float32, tag="lnt2")
            nc.scalar.activation(lnt2[:], n2t_ps[:], AF.Ln)
            ntp = cp.tile([F, 1], DT.float32, tag="ntp")
            nc.scalar.activation(ntp[:], lnt2[:], AF.Exp, scale=0.5)
            lgt = cp.tile([F, 1], DT.float32, tag="lgt")
            nc.scalar.activation(lgt[:], ntp[:], AF.Ln, bias=1.0)
            aux_ps = ps.tile([1, 1], DT.float32, tag="mm")
            nc.tensor.matmul(aux_ps[:], w32, lgt[0:K, :], start=True, stop=True)
            aux_sb = cp.tile([1, 1], DT.float32, tag="auxsb")
            nc.scalar.copy(aux_sb[:], aux_ps[:])
            nc.sync.dma_start(aux_d, aux_sb[:])

            # ---- pass 1: ln_mag + fnorm partials ----
            acc = cp.tile([F, 1], DT.float32, tag="acc")
            for j in range(NPAIR):
                c0, c1 = j * PAIR, (j + 1) * PAIR
                p_ri = _emit_pair_matmuls(nc, ps, s16_t, l16_r, l16_i, j)
                sq = wp.tile([F, 2 * PAIR], DT.float32, tag="sq")
                rj = wp.tile([F, 1], DT.float32, tag="rj")
                nc.scalar.activation(sq[:], p_ri[:], AF.Square, accum_out=rj[:])
                m = wp.tile([F, PAIR], DT.float32, tag="m")
                nc.vector.tensor_add(m[:], sq[:, 0:PAIR], sq[:, PAIR : 2 * PAIR])
                ln_t = wp.tile([F, PAIR], DT.float32, tag="lnt")
                nc.scalar.activation(
                    ln_t[:], m[:], AF.Ln, bias=epsb, scale=1.0 / (P * P)
                )
                lnm = wp.tile([F, PAIR], DT.float32, tag="lnm")
                nc.vector.tensor_scalar(
                    out=lnm[:], in0=ln_t[:], scalar1=biasf[:, 0:1],
                    scalar2=1.0 / AMPLITUDE_STD, op0=ALU.add, op1=ALU.mult,
                )
                nc.sync.dma_start(ln_d[:, c0:c1], lnm[:])
                if j == 0:
                    nc.vector.tensor_copy(acc[:], rj[:])
                else:
                    nc.vector.tensor_add(acc[:], acc[:], rj[:])

            # ---- 1/fnorm ----
            fcl = cp.tile([F, 1], DT.float32, tag="fcl")
            nc.vector.tensor_scalar_max(fcl[:], acc[:], F32_EPS * P * P)
            lnf = cp.tile([F, 1], DT.float32, tag="lnf")
            nc.scalar.activation(lnf[:], fcl[:], AF.Ln)
            recf = cp.tile([F, 1], DT.float32, tag="recf")
            nc.scalar.activation(recf[:], lnf[:], AF.Exp, scale=-0.5)

            # ---- pass 2: normed field ----
            for j in range(NPAIR):
                c0, c1 = j * PAIR, (j + 1) * PAIR
                p_ri = _emit_pair_matmuls(nc, ps, s16_t, l16_r, l16_i, j)
                fld = wp.tile([F, 2 * PAIR], DT.float32, tag="fld")
                nc.vector.tensor_scalar(
                    out=fld[:, 0:PAIR], in0=p_ri[:, 0:PAIR], scalar1=recf[:],
                    scalar2=None, op0=ALU.mult,
                )
                nc.scalar.activation(
                    fld[:, PAIR : 2 * PAIR], p_ri[:, PAIR : 2 * PAIR],
                    AF.Copy, scale=recf[:],
                )
                nc.sync.dma_start(
                    fld_d[:, :, c0:c1],
                    fld[:].rearrange("p (c n) -> p c n", c=2),
                )

    _dedup_ldweights(nc)
    _split_sync_waits(nc)
    return nc


_NC = None


def _get_nc():
    global _NC
    if _NC is None:
        _NC = build_nc()
    return _NC


def _make_in_maps(mode_shapes, mode_responses, freqs, bias_table):
    mode_shapes = np.ascontiguousarray(np.asarray(mode_shapes, dtype=np.float32))
    mode_responses = np.ascontiguousarray(np.asarray(mode_responses, dtype=np.float32))
    freqs = np.asarray(freqs, dtype=np.float32)
    bias_table = np.asarray(bias_table, dtype=np.float32)

    # frequency bias interpolation (host, f32 arithmetic to match the f32 ref)
    n = bias_table.shape[0]
    idx = np.clip(
        (freqs + np.float32(1.0)) * np.float32(0.5) * np.float32(n - 1),
        np.float32(0.0),
        np.float32(n - 1),
    ).astype(np.float32)
    i0 = np.floor(idx).astype(np.int32)
    i1 = np.minimum(i0 + 1, n - 1)
    w = (idx - i0.astype(np.float32)).astype(np.float32)
    bias = (
        bias_table[i0] * (np.float32(1.0) - w) + bias_table[i1] * w
    ).astype(np.float32)  # (B, F)

    ps4 = np.zeros((K, F), dtype=np.float32)
    for k in range(K):
        ps4[k, k % P :: P] = 1.0
    sgn = np.concatenate([np.full((P, 1), 1.0), np.full((P, 1), -1.0)]).astype(
        np.float32
    )
    w32 = np.concatenate([np.full((P, 1), 1.0 / P), np.zeros((P, 1))]).astype(
        np.float32
    )

    in_maps = []
    for b in range(B):
        s_b = mode_shapes[b].reshape(K, WH)
        s16 = np.ascontiguousarray(np.tile(s_b.astype(np.float16), (2, 1)))
        tt = mode_responses[b].transpose(1, 2, 0).reshape(K, F)
        tt2 = np.concatenate([tt[P:], tt[:P]], axis=0)
        cst64 = np.concatenate([tt, tt2, ps4, sgn, w32], axis=1)
        cst = np.ascontiguousarray(np.tile(cst64, (2, 1)), dtype=np.float32)
        biasf = np.stack(
            [bias[b], np.full((F,), EPS, dtype=np.float32)], axis=1
        ).astype(np.float32)
        in_maps.append({"s16": s16, "cst": cst, "biasf": biasf})
    return in_maps


def run_cores(mode_shapes, mode_responses, freqs, bias_table, trace=False, tmpdir=None):
    nc = _get_nc()
    in_maps = _make_in_maps(mode_shapes, mode_responses, freqs, bias_table)
    res = run_bass_kernel_spmd(
        nc, in_maps, core_ids=list(range(B)), trace=trace, tmpdir=tmpdir
    )
    ln = np.stack([res.results[c]["ln"].reshape(F, W, H) for c in range(B)])
    fld = np.stack([res.results[c]["fld"].reshape(F, 2, W, H) for c in range(B)])
    aux = np.float32(np.mean([res.results[c]["aux"][0, 0] for c in range(B)]))
    return (ln, fld, aux), res


def kernel(mode_shapes, mode_responses, freqs, bias_table):
    (ln, fld, aux), _ = run_cores(mode_shapes, mode_responses, freqs, bias_table)
    return ln, fld, aux


# revision 9
# speedup vs baseline: 1.0525x; 1.0525x over previous
"""Trainium2 Bass kernel for nn_ModeONet: complex mode superposition.

Sharding: data-parallel over B — each of the 8 NeuronCores processes one
batch sample (its own mode_shapes/mode_responses/freqs slice).

Per-core math (b fixed; S = mode_shapes[b] as (64, 9216) with rows
[s_r(32p); s_i(32p)], T = mode_responses[b] transposed to (64, 128)):
  norm2[p]   = sum_{c,wh} S[c*32+p]^2
  v[k]       = 1/(sqrt(norm2)+1e-8) / 32            (1/P folded in)
  lhsT_r     = [ t_r * v ; -t_i * v ]   (64,128)
  lhsT_i     = [ t_i * v ;  t_r * v ]
  r = lhsT_r.T @ S, i = lhsT_i.T @ S                (TensorE)
  mag2 = r^2 + i^2; ln = (Ln(mag2+1e-8)+bias_f)/4.2
  fnorm = sqrt(max(sum_wh mag2, f32eps)); field = (r, i) * (1/fnorm)
  aux_b = mean_p ln(sqrt(sum_{f,c} T^2) + 1)        (host averages over b)

Matmuls run in bf16 with a hi/lo split for ~fp32 accuracy, 2 matmuls per
product: S is host-split into S_hi/S_lo stacked on partitions
(s2 = [S_hi; S_lo], (128, WH) bf16); lhsT is split on device.  Then
  MM1 (K=64):  a_hi.T @ S_hi
  MM2 (K=128): [a_lo; a_hi].T @ [S_hi; S_lo] = a_lo@S_hi + a_hi@S_lo
accumulated in PSUM (the dropped a_lo@S_lo term is ~2^-32).  The mode
norm is computed from s2 directly: sum(hi^2)+sum(lo^2) — the missing
2*sum(hi*lo) cross term is a random-rounding sum, relative size ~4e-6.

r and i for one column-pair live in ONE (128, 2048) PSUM tile so a single
ScalarE Square covers both, and its accum_out row-sum IS the fnorm
partial.  Pass 2 recomputes the matmuls (TensorE has slack) so the
mandatory PSUM->SBUF move doubles as the 1/fnorm scaling.  The bias
interpolation over the 250-entry table (1K elements) is host-computed.

The walrus in this environment caps sync waits at 1 per instruction,
while Tile emits as many as needed; _split_sync_waits hoists the excess
onto preceding same-engine NoOps (equivalent under per-engine program
order).
"""
import numpy as np
import ml_dtypes
import concourse.bass as bass
import concourse.bass_utils as _bu
import concourse.tile as tile
from concourse import mybir
from concourse.bass_utils import run_bass_kernel_spmd

def _dedup_ldweights(nc):
    """Drop InstLdweights that reload the stationary operand already resident
    in the PE array (identical tensor/offset/pattern as the previous LDW on
    the PE stream).  The pipeline's walrus runs with --enable-ldw-opt=false,
    so repeated weights would otherwise reload every matmul.  A dropped LDW
    that carries sync info is replaced by a NoOp (keeps waits/updates)."""
    n = 0
    for fn in nc.m.functions:
        for blk in fn.blocks:
            out = []
            last_sig = None
            for inst in blk.instructions:
                if getattr(inst, "engine", None) != mybir.EngineType.PE:
                    out.append(inst)
                    continue
                tname = type(inst).__name__
                if tname == "InstLdweights":
                    ap = inst.ins[0]
                    bap = getattr(ap, "bass_ap", None)
                    sig = (
                        (bap.tensor.name, bap.offset, str(bap.ap))
                        if bap is not None
                        else None
                    )
                    if sig is not None and sig == last_sig:
                        si = inst.sync_info
                        has_sync = si is not None and (si.on_wait or si.on_update)
                        if has_sync:
                            out.append(
                                mybir.InstNoOp(
                                    name=f"{inst.name}-ldwdedup",
                                    engine=inst.engine,
                                    ins=[],
                                    outs=[],
                                    bass_nofuse=True,
                                    sync_info=si,
                                )
                            )
                        n += 1
                        continue
                    last_sig = sig
                elif tname == "InstMatmult":
                    pass  # matmuls don't clobber the weight buffer
                else:
                    last_sig = None  # anything else on PE: be conservative
                out.append(inst)
            blk.instructions = out
    return n

B, F, P, W, H = 8, 128, 32, 96, 96
WH = W * H                     # 9216
K = 2 * P                      # 64 contraction rows (real; imag)
PAIR = 1024                    # column-pair width (r|i -> one 4-bank PSUM tile)
NPAIR = WH // PAIR             # 9
NORM_CHUNK = WH // 4           # 2304
AMPLITUDE_STD = 4.2
EPS = 1e-8
F32_EPS = float(np.finfo(np.float32).eps)
AF = mybir.ActivationFunctionType
ALU = mybir.AluOpType
DT = mybir.dt

# cst128 column layout
C_TT, C_TT2, C_PS4, C_SGN, C_W32 = 0, 128, 256, 384, 385


def _split_sync_waits(nc, inst_cap=1):
    """Hoist excess sync waits onto preceding same-engine NoOps."""
    for fn in nc.m.functions:
        for blk in fn.blocks:
            out = []
            for inst in blk.instructions:
                si = inst.sync_info
                waits = list(si.on_wait) if si is not None and si.on_wait else []
                if len(waits) > inst_cap and not isinstance(
                    inst, mybir.InstCall | mybir.InstUnconditionalBranch
                ):
                    keep = waits[-inst_cap:]
                    for ci, w in enumerate(waits[:-inst_cap]):
                        out.append(
                            mybir.InstNoOp(
                                name=f"{inst.name}-wsplit{ci}",
                                engine=inst.engine,
                                ins=[],
                                outs=[],
                                bass_nofuse=True,
                                sync_info=mybir.SyncInfo(on_wait=[w], on_update=[]),
                            )
                        )
                    inst.sync_info = mybir.SyncInfo(
                        on_wait=keep,
                        on_update=list(si.on_update) if si.on_update else [],
                    )
                out.append(inst)
            blk.instructions = out


def _emit_pair_matmuls(nc, ps, s16_t, l16_r, l16_i, j):
    """4 fp16 matmuls for column pair j -> one (128, 2048) PSUM tile [r | i].

    The two 512-column chunks of a pair run CONCURRENTLY as 2-way K=64
    row-tiles: chunk 0 on PE rows 0-63, chunk 1 on rows 64-127 (s16 and the
    coefficients are duplicated across partition halves)."""
    p_ri = ps.tile([F, 2 * PAIR], DT.float32, tag="mm")
    for base, l16 in ((0, l16_r), (PAIR, l16_i)):
        for h in range(2):
            cols = slice(j * PAIR + h * 512, j * PAIR + (h + 1) * 512)
            reg = slice(base + h * 512, base + (h + 1) * 512)
            rows = slice(h * K, (h + 1) * K)
            nc.tensor.matmul(
                p_ri[:, reg], l16[rows, :], s16_t[rows, cols],
                start=True, stop=True,
            )
    return p_ri


def build_nc():
    nc = bass.Bass("TRN2", target_bir_lowering=False, debug=False)
    s16_d = nc.dram_tensor("s16", (2 * K, WH), DT.float16, kind="ExternalInput").ap()
    cst_d = nc.dram_tensor("cst", (2 * K, 386), DT.float32, kind="ExternalInput").ap()
    biasf_d = nc.dram_tensor("biasf", (F, 2), DT.float32, kind="ExternalInput").ap()
    ln_d = nc.dram_tensor("ln", (F, WH), DT.float32, kind="ExternalOutput").ap()
    fld_d = nc.dram_tensor("fld", (F, 2, WH), DT.float32, kind="ExternalOutput").ap()
    aux_d = nc.dram_tensor("aux", (1, 1), DT.float32, kind="ExternalOutput").ap()

    with tile.TileContext(nc) as tc:
        with (
            tc.tile_pool(name="cp", bufs=1) as cp,
            tc.tile_pool(name="wp", bufs=4) as wp,
            tc.tile_pool(name="np2", bufs=2) as np2,
            tc.tile_pool(name="ps", bufs=2, space="PSUM") as ps,
        ):
            s16_t = cp.tile([2 * K, WH], DT.float16, tag="s16")
            for q in range(4):
                cs = slice(q * NORM_CHUNK, (q + 1) * NORM_CHUNK)
                nc.sync.dma_start(s16_t[0:K, cs], s16_d[0:K, cs])
            cst = cp.tile([2 * K, 386], DT.float32, tag="cst")
            nc.sync.dma_start(cst[:], cst_d)
            biasf = cp.tile([F, 2], DT.float32, tag="biasf")
            nc.sync.dma_start(biasf[:], biasf_d)
            nc.sync.dma_start(s16_t[K : 2 * K, :], s16_d[K : 2 * K, :])

            tt = cst[0:K, C_TT : C_TT + 128]
            tt2 = cst[0:K, C_TT2 : C_TT2 + 128]
            ps4_128 = cst[:, C_PS4 : C_PS4 + 128]
            ps4_64 = cst[0:K, C_PS4 : C_PS4 + 128]
            sgn = cst[:, C_SGN : C_SGN + 1]
            w32 = cst[0:K, C_W32 : C_W32 + 1]
            epsb = biasf[:, 1:2]

            # ---- mode norms: red[k] = sum_wh s2[k]^2 over all 128 rows ----
            red4 = []
            for q in range(4):
                cs = slice(q * NORM_CHUNK, (q + 1) * NORM_CHUNK)
                scr = np2.tile([K, NORM_CHUNK], DT.float32, tag="nscr")
                rq = cp.tile([K, 1], DT.float32, tag=f"red{q}")
                if q > 0:
                    nc.scalar.activation(
                        scr[:], s16_t[0:K, cs], AF.Square, accum_out=rq[:]
                    )
                else:
                    nc.vector.tensor_mul(scr[:], s16_t[0:K, cs], s16_t[0:K, cs])
                    nc.vector.tensor_reduce(
                        rq[:], scr[:], axis=mybir.AxisListType.X, op=ALU.add
                    )
                red4.append(rq)
            r01 = cp.tile([K, 1], DT.float32, tag="r01")
            nc.vector.tensor_add(r01[:], red4[0][:], red4[1][:])
            r23 = cp.tile([K, 1], DT.float32, tag="r23")
            nc.vector.tensor_add(r23[:], red4[2][:], red4[3][:])
            red_s = cp.tile([K, 1], DT.float32, tag="red_s")
            nc.vector.tensor_add(red_s[:], r01[:], r23[:])

            # pair-sum across (real, imag) rows k = p mod 32 via PE (K=64)
            n2_ps = ps.tile([F, 1], DT.float32, tag="mm")
            nc.tensor.matmul(n2_ps[:], ps4_64, red_s[:], start=True, stop=True)
            # v = n2^(-1/2) = Exp(-0.5*Ln(n2)); the reference's +1e-8 on the
            # norm is ~1e-10 relative here (norm ~ O(100)) and is dropped.
            lnn = cp.tile([F, 1], DT.float32, tag="lnn")
            nc.scalar.activation(lnn[:], n2_ps[:], AF.Ln)
            v = cp.tile([F, 1], DT.float32, tag="v")
            nc.scalar.activation(v[:], lnn[:], AF.Exp, scale=-0.5)
            vr = cp.tile([2 * K, 1], DT.float32, tag="vr")
            nc.vector.tensor_mul(vr[:], v[:], sgn)

            # lhsT in fp32 on all 128 partitions (rows duplicated), then split
            lr32 = cp.tile([2 * K, F], DT.float32, tag="lr32")
            nc.vector.tensor_scalar(
                out=lr32[:], in0=cst[:, C_TT : C_TT + 128], scalar1=vr[:],
                scalar2=None, op0=ALU.mult,
            )
            li32 = cp.tile([2 * K, F], DT.float32, tag="li32")
            nc.vector.tensor_scalar(
                out=li32[:], in0=cst[:, C_TT2 : C_TT2 + 128], scalar1=v[:],
                scalar2=None, op0=ALU.mult,
            )
            l16_r = cp.tile([2 * K, F], DT.float16, tag="l16r")
            nc.vector.tensor_copy(l16_r[:], lr32[:])
            l16_i = cp.tile([2 * K, F], DT.float16, tag="l16i")
            nc.vector.tensor_copy(l16_i[:], li32[:])

            # ---- aux sparsity scalar ----
            tsq = cp.tile([K, F], DT.float32, tag="tsq")
            redt = cp.tile([K, 1], DT.float32, tag="redt")
            nc.scalar.activation(tsq[:], tt, AF.Square, accum_out=redt[:])
            n2t_ps = ps.tile([F, 1], DT.float32, tag="mm")
            nc.tensor.matmul(n2t_ps[:], ps4_64, redt[:], start=True, stop=True)
            lnt2 = cp.tile([F, 1], DT.float32, tag="lnt2")
            nc.scalar.activation(lnt2[:], n2t_ps[:], AF.Ln)
            ntp = cp.tile([F, 1], DT.float32, tag="ntp")
            nc.scalar.activation(ntp[:], lnt2[:], AF.Exp, scale=0.5)
            lgt = cp.tile([F, 1], DT.float32, tag="lgt")
            nc.scalar.activation(lgt[:], ntp[:], AF.Ln, bias=1.0)
            aux_ps = ps.tile([1, 1], DT.float32, tag="mm")
            nc.tensor.matmul(aux_ps[:], w32, lgt[0:K, :], start=True, stop=True)
            aux_sb = cp.tile([1, 1], DT.float32, tag="auxsb")
            nc.scalar.copy(aux_sb[:], aux_ps[:])
            nc.sync.dma_start(aux_d, aux_sb[:])

            # ---- pass 1: ln_mag + fnorm partials ----
            acc = cp.tile([F, 1], DT.float32, tag="acc")
            for j in range(NPAIR):
                c0, c1 = j * PAIR, (j + 1) * PAIR
                p_ri = _emit_pair_matmuls(nc, ps, s16_t, l16_r, l16_i, j)
                sq = wp.tile([F, 2 * PAIR], DT.float32, tag="sq")
                rj = wp.tile([F, 1], DT.float32, tag="rj")
                nc.scalar.activation(sq[:], p_ri[:], AF.Square, accum_out=rj[:])
                m = wp.tile([F, PAIR], DT.float32, tag="m")
                nc.vector.tensor_add(m[:], sq[:, 0:PAIR], sq[:, PAIR : 2 * PAIR])
                ln_t = wp.tile([F, PAIR], DT.float32, tag="lnt")
                nc.scalar.activation(
                    ln_t[:], m[:], AF.Ln, bias=epsb, scale=1.0 / (P * P)
                )
                lnm = wp.tile([F, PAIR], DT.float32, tag="lnm")
                nc.vector.tensor_scalar(
                    out=lnm[:], in0=ln_t[:], scalar1=biasf[:, 0:1],
                    scalar2=1.0 / AMPLITUDE_STD, op0=ALU.add, op1=ALU.mult,
                )
                nc.sync.dma_start(ln_d[:, c0:c1], lnm[:])
                if j == 0:
                    nc.vector.tensor_copy(acc[:], rj[:])
                else:
                    nc.vector.tensor_add(acc[:], acc[:], rj[:])

            # ---- 1/fnorm ----
            fcl = cp.tile([F, 1], DT.float32, tag="fcl")
            nc.vector.tensor_scalar_max(fcl[:], acc[:], F32_EPS * P * P)
            lnf = cp.tile([F, 1], DT.float32, tag="lnf")
            nc.scalar.activation(lnf[:], fcl[:], AF.Ln)
            recf = cp.tile([F, 1], DT.float32, tag="recf")
            nc.scalar.activation(recf[:], lnf[:], AF.Exp, scale=-0.5)

            # ---- pass 2: normed field ----
            for j in range(NPAIR):
                c0, c1 = j * PAIR, (j + 1) * PAIR
                p_ri = _emit_pair_matmuls(nc, ps, s16_t, l16_r, l16_i, j)
                fld = wp.tile([F, 2 * PAIR], DT.float32, tag="fld")
                nc.vector.tensor_scalar(
                    out=fld[:, 0:PAIR], in0=p_ri[:, 0:PAIR], scalar1=recf[:],
                    scalar2=None, op0=ALU.mult,
                )
                nc.scalar.activation(
                    fld[:, PAIR : 2 * PAIR], p_ri[:, PAIR : 2 * PAIR],
                    AF.Copy, scale=recf[:],
                )
                nc.sync.dma_start(
                    fld_d[:, :, c0:c1],
                    fld[:].rearrange("p (c n) -> p c n", c=2),
                )

    _dedup_ldweights(nc)
    _split_sync_waits(nc)
    return nc


_NC = None


def _get_nc():
    global _NC
    if _NC is None:
        _NC = build_nc()
    return _NC


def _make_in_maps(mode_shapes, mode_responses, freqs, bias_table):
    mode_shapes = np.ascontiguousarray(np.asarray(mode_shapes, dtype=np.float32))
    mode_responses = np.ascontiguousarray(np.asarray(mode_responses, dtype=np.float32))
    freqs = np.asarray(freqs, dtype=np.float32)
    bias_table = np.asarray(bias_table, dtype=np.float32)

    # frequency bias interpolation (host, f32 arithmetic to match the f32 ref)
    n = bias_table.shape[0]
    idx = np.clip(
        (freqs + np.float32(1.0)) * np.float32(0.5) * np.float32(n - 1),
        np.float32(0.0),
        np.float32(n - 1),
    ).astype(np.float32)
    i0 = np.floor(idx).astype(np.int32)
    i1 = np.minimum(i0 + 1, n - 1)
    w = (idx - i0.astype(np.float32)).astype(np.float32)
    bias = (
        bias_table[i0] * (np.float32(1.0) - w) + bias_table[i1] * w
    ).astype(np.float32)  # (B, F)

    ps4 = np.zeros((K, F), dtype=np.float32)
    for k in range(K):
        ps4[k, k % P :: P] = 1.0
    sgn = np.concatenate([np.full((P, 1), 1.0), np.full((P, 1), -1.0)]).astype(
        np.float32
    )
    w32 = np.concatenate([np.full((P, 1), 1.0 / P), np.zeros((P, 1))]).astype(
        np.float32
    )

    in_maps = []
    for b in range(B):
        s_b = mode_shapes[b].reshape(K, WH)
        s16 = np.ascontiguousarray(np.tile(s_b.astype(np.float16), (2, 1)))
        tt = mode_responses[b].transpose(1, 2, 0).reshape(K, F)
        tt2 = np.concatenate([tt[P:], tt[:P]], axis=0)
        cst64 = np.concatenate([tt, tt2, ps4, sgn, w32], axis=1)
        cst = np.ascontiguousarray(np.tile(cst64, (2, 1)), dtype=np.float32)
        biasf = np.stack(
            [bias[b], np.full((F,), EPS, dtype=np.float32)], axis=1
        ).astype(np.float32)
        in_maps.append({"s16": s16, "cst": cst, "biasf": biasf})
    return in_maps


def run_cores(mode_shapes, mode_responses, freqs, bias_table, trace=False, tmpdir=None):
    nc = _get_nc()
    in_maps = _make_in_maps(mode_shapes, mode_responses, freqs, bias_table)
    res = run_bass_kernel_spmd(
        nc, in_maps, core_ids=list(range(B)), trace=trace, tmpdir=tmpdir
    )
    ln = np.stack([res.results[c]["ln"].reshape(F, W, H) for c in range(B)])
    fld = np.stack([res.results[c]["fld"].reshape(F, 2, W, H) for c in range(B)])
    aux = np.float32(np.mean([res.results[c]["aux"][0, 0] for c in range(B)]))
    return (ln, fld, aux), res


def kernel(mode_shapes, mode_responses, freqs, bias_table):
    (ln, fld, aux), _ = run_cores(mode_shapes, mode_responses, freqs, bias_table)
    return ln, fld, aux
